# revision 12
# baseline (speedup 1.0000x reference)
"""Trainium2 Bass kernel for nn_CHSLoss2 (topk_masking CHS loss).

Self-contained: takes FULL inputs, shards batch over 8 NeuronCores,
runs one Bass/Tile kernel per core, sums the per-core partial losses.

Math (per batch row b, with n=3 outputs, w = weight, d_i = out_i - dmap):
  loss = sum_{i<j} [ sum d_i^2 + sum mask_i * (w d_j) * (w d_j - 2 d_i) ]
  mask_i = err_i >= v_min(i)  with v_min = num-th largest of err_i = |d_i|.
A threshold t with count(err_i >= t) == num produces the identical mask,
so we find t per (image, i) by a fixed number of regula-falsi iterations
on the exact count function, bracketed by Gaussian-moment estimates.
"""

import math

import numpy as np

# ---- problem geometry (hardcoded per the task spec) ----
N_CORES = 8
B, C, H, W = 32, 1, 192, 192
HW = H * W                     # 36864 elements per image
SIZE = 8
GH, GW = H * SIZE, W * SIZE    # 1536 x 1536
MAX_NOISY_RATIO = 0.1
MAX_WEIGHT_RATIO = 1.0

B_LOC = B // N_CORES           # 4 images per core
P = 128                        # SBUF partitions
FREE = B_LOC * HW // P         # 1152 f32 per partition (canonical layout)
IMG_PARTS = P // B_LOC         # 32 partitions per image
NCHUNK = 8                     # pooling chunks of 96 pooled rows per core
GT_ROWS = B_LOC * GH           # 6144 gt rows per core

R_ITERS = 7                    # regula-falsi refinement iterations
Z_MARGIN = 0.45                # bracket half-width in sigmas around the quantile
POOL_PREC = "f32r"             # "f32r" (fast PE path) or "f32"

_CACHE = {}


def _norm_ppf(p):
    """Acklam's rational approximation of the standard normal inverse CDF."""
    a = [-3.969683028665376e+01, 2.209460984245205e+02, -2.759285104469687e+02,
         1.383577518672690e+02, -3.066479806614716e+01, 2.506628277459239e+00]
    b = [-5.447609879822406e+01, 1.615858368580409e+02, -1.556989798598866e+02,
         6.680131188771972e+01, -1.328068155288572e+01]
    c = [-7.784894002430293e-03, -3.223964580411365e-01, -2.400758277161838e+00,
         -2.549732539343734e+00, 4.374664141464968e+00, 2.938163982698783e+00]
    d = [7.784695709041462e-03, 3.224671290700398e-01, 2.445134137142996e+00,
         3.754408661907416e+00]
    plow, phigh = 0.02425, 1 - 0.02425
    if p < plow:
        q = math.sqrt(-2 * math.log(p))
        return (((((c[0] * q + c[1]) * q + c[2]) * q + c[3]) * q + c[4]) * q + c[5]) / \
               ((((d[0] * q + d[1]) * q + d[2]) * q + d[3]) * q + 1)
    if p > phigh:
        q = math.sqrt(-2 * math.log(1 - p))
        return -(((((c[0] * q + c[1]) * q + c[2]) * q + c[3]) * q + c[4]) * q + c[5]) / \
               ((((d[0] * q + d[1]) * q + d[2]) * q + d[3]) * q + 1)
    q = p - 0.5
    r = q * q
    return (((((a[0] * r + a[1]) * r + a[2]) * r + a[3]) * r + a[4]) * r + a[5]) * q / \
           (((((b[0] * r + b[1]) * r + b[2]) * r + b[3]) * r + b[4]) * r + 1)


def _host_consts():
    p = np.arange(P)
    ind4 = (p[:, None] // IMG_PARTS == np.arange(B_LOC)[None, :]).astype(np.float32)
    bcast4 = ind4.T.copy()                      # [4, 128]
    ones1 = np.ones((P, 1), np.float32)
    # ind96[j]: [128, 96] stationary for pooling sub-slab j; out row m = 16*j + p//8
    ind96 = np.zeros((6, P, 96), np.float32)
    for j in range(6):
        ind96[j, p, 16 * j + p // 8] = 1.0
    return ind4, bcast4, ones1, ind96


def _build(num, weight):
    """Trace + compile the per-core Bass kernel. Returns compiled nc."""
    from contextlib import ExitStack

    from concourse import bacc
    import concourse.mybir as mybir
    import concourse.tile as tile

    f32 = mybir.dt.float32
    f32r = mybir.dt.float32r
    ALU = mybir.AluOpType
    AX = mybir.AxisListType
    AF = mybir.ActivationFunctionType

    zq = _norm_ppf(1.0 - num / float(HW))
    z_lo = zq - Z_MARGIN
    z_hi = zq + Z_MARGIN
    inv_n = 1.0 / float(HW)
    k = float(num)
    w = float(weight)

    nc = bacc.Bacc("TRN2", target_bir_lowering=False, debug=False)

    gt = nc.dram_tensor("gt", [GT_ROWS, GW], f32, kind="ExternalInput").ap()
    outs_d = [nc.dram_tensor(f"out{i}", [P, FREE], f32, kind="ExternalInput").ap()
              for i in range(3)]
    ind4_d = nc.dram_tensor("ind4", [P, B_LOC], f32, kind="ExternalInput").ap()
    bcast4_d = nc.dram_tensor("bcast4", [B_LOC, P], f32, kind="ExternalInput").ap()
    ones1_d = nc.dram_tensor("ones1", [P, 1], f32, kind="ExternalInput").ap()
    ind96_d = nc.dram_tensor("ind96", [6, P, 96], f32, kind="ExternalInput").ap()
    dmap_scr_d = nc.dram_tensor("dmap_scratch", [6 * P, W], f32).ap()
    loss_d = nc.dram_tensor("loss", [1, 1], f32, kind="ExternalOutput").ap()
    dbg_d = nc.dram_tensor("dbg", [B_LOC, 24], f32, kind="ExternalOutput").ap()

    with tile.TileContext(nc) as tc, ExitStack() as ctx:
        const_p = ctx.enter_context(tc.tile_pool(name="const", bufs=1))
        persist = ctx.enter_context(tc.tile_pool(name="persist", bufs=1))
        gt_p = ctx.enter_context(tc.tile_pool(name="gtin", bufs=2))
        stage_p = ctx.enter_context(tc.tile_pool(name="stage", bufs=3))
        scratch = ctx.enter_context(tc.tile_pool(name="scratch", bufs=1))
        tiny = ctx.enter_context(tc.tile_pool(name="tiny", bufs=3))
        psum_pool = ctx.enter_context(tc.tile_pool(name="pp", bufs=2, space="PSUM"))
        psum_sm = ctx.enter_context(tc.tile_pool(name="ps", bufs=2, space="PSUM"))

        # ---- constants ----
        c_ind4 = const_p.tile([P, B_LOC], f32, name="ind4", tag="ind4")
        nc.sync.dma_start(c_ind4[:], ind4_d[:])
        c_bc4 = const_p.tile([B_LOC, P], f32, name="bc4", tag="bc4")
        nc.sync.dma_start(c_bc4[:], bcast4_d[:])
        c_ones = const_p.tile([P, 1], f32, name="ones1", tag="ones1")
        nc.sync.dma_start(c_ones[:], ones1_d[:])
        ind_dt = f32r if POOL_PREC == "f32r" else f32
        c_ind96 = const_p.tile([P, 6, 96], ind_dt, name="ind96", tag="ind96")
        nc.gpsimd.dma_start(c_ind96[:], ind96_d.rearrange("j p m -> p j m"))

        # ---- load outs into canonical layout (contiguous reshape) ----
        outs_sb = []
        for i in range(3):
            t = persist.tile([P, FREE], f32, name=f"o{i}", tag=f"o{i}")
            nc.scalar.dma_start(t[:], outs_d[i][:])
            outs_sb.append(t)

        dmap = persist.tile([P, FREE], f32, name="dmap", tag="dmap")

        # ---- pooling: 8 chunks of 96 pooled rows ----
        for cix in range(NCHUNK):
            gtt = gt_p.tile([P, 6, GW], ind_dt, name="gtt", tag="gtt")
            src = gt[768 * cix: 768 * (cix + 1), :].rearrange("(j p) w -> p j w", p=P)
            nc.gpsimd.dma_start(gtt[:], src)
            ps = psum_pool.tile([96, GW], f32, name="pool", tag="pool")
            for n in range(3):
                for j in range(6):
                    lhsT = c_ind96[:, j, :]
                    rhs = gtt[:, j, 512 * n: 512 * (n + 1)]
                    nc.tensor.matmul(ps[:, 512 * n: 512 * (n + 1)], lhsT, rhs,
                                     start=(j == 0), stop=(j == 5))
            stg = stage_p.tile([96, W], f32, name="stg", tag="stg")
            nc.vector.tensor_reduce(stg[:], ps[:].rearrange("p (a b) -> p a b", b=SIZE),
                                    axis=AX.X, op=ALU.add)
            nc.gpsimd.dma_start(dmap_scr_d[96 * cix: 96 * (cix + 1), :], stg[:])

        # gather pooled rows into canonical layout: partition p <- rows 6p..6p+5
        nc.sync.dma_start(dmap[:].rearrange("p (m w) -> p m w", m=6),
                          dmap_scr_d.rearrange("(p m) w -> p m w", m=6))

        # ---- d_i, err_i, per-partition moments ----
        d_sb = [persist.tile([P, FREE], f32, name=f"d{i}", tag=f"d{i}") for i in range(3)]
        e_sb = [persist.tile([P, FREE], f32, name=f"e{i}", tag=f"e{i}") for i in range(3)]
        stats = persist.tile([P, 8], f32, name="stats", tag="stats")     # cols 0-2 S1, 3-5 S2
        sq_scr = scratch.tile([P, FREE], f32, name="sq", tag="sq")
        msk_scr = scratch.tile([P, FREE], f32, name="msk", tag="msk")
        for i in range(3):
            nc.vector.tensor_sub(d_sb[i][:], outs_sb[i][:], dmap[:])
            nc.scalar.activation(e_sb[i][:], d_sb[i][:], AF.Abs)
            nc.vector.tensor_reduce(stats[:, i: i + 1], e_sb[i][:],
                                    axis=AX.X, op=ALU.add)
            nc.scalar.activation(sq_scr[:], d_sb[i][:], AF.Square)
            nc.vector.tensor_reduce(stats[:, 3 + i: 4 + i], sq_scr[:],
                                    axis=AX.X, op=ALU.add)

        # ---- per-image moments -> bracket [lo, hi] ----
        ps_sm = psum_sm.tile([P, 8], f32, name="sm", tag="sm")
        nc.tensor.matmul(ps_sm[:B_LOC, 0:6], c_ind4[:], stats[:, 0:6],
                         start=True, stop=True)
        st4 = tiny.tile([B_LOC, 6], f32, name="st4", tag="st4")
        nc.vector.tensor_copy(st4[:], ps_sm[:B_LOC, 0:6])
        mu = tiny.tile([B_LOC, 3], f32, name="mu", tag="mu")
        nc.vector.tensor_scalar_mul(mu[:], st4[:, 0:3], inv_n)
        e2 = tiny.tile([B_LOC, 3], f32, name="e2", tag="e2")
        nc.vector.tensor_scalar_mul(e2[:], st4[:, 3:6], inv_n)
        mu2 = tiny.tile([B_LOC, 3], f32, name="mu2", tag="mu2")
        nc.vector.tensor_mul(mu2[:], mu[:], mu[:])
        var = tiny.tile([B_LOC, 3], f32, name="var", tag="var")
        nc.vector.tensor_sub(var[:], e2[:], mu2[:])
        sig = tiny.tile([B_LOC, 3], f32, name="sig", tag="sig")
        nc.scalar.activation(sig[:], var[:], AF.Sqrt)
        lohi = tiny.tile([B_LOC, 6], f32, name="lohi", tag="lohi")
        nc.vector.scalar_tensor_tensor(lohi[:, 0:3], sig[:], z_lo, mu[:],
                                       ALU.mult, ALU.add)
        nc.vector.scalar_tensor_tensor(lohi[:, 3:6], sig[:], z_hi, mu[:],
                                       ALU.mult, ALU.add)

        def bcast(src_ap, width, tag):
            """[4, width] -> [128, width] per-image broadcast via PE."""
            pb = psum_sm.tile([P, 8], f32, name="sm", tag="sm")
            nc.tensor.matmul(pb[:, 0:width], c_bc4[:], src_ap, start=True, stop=True)
            out = persist.tile([P, 8], f32, name=tag, tag=tag)
            nc.scalar.copy(out[:, 0:width], pb[:, 0:width])
            return out

        def count3(thr_cols, cnt_cols):
            """cnt_cols[:, i] = per-partition count of e_i >= thr_cols[:, i]."""
            for i in range(3):
                nc.vector.tensor_scalar(msk_scr[:], e_sb[i][:],
                                        thr_cols[:, i: i + 1], None,
                                        ALU.is_ge, ALU.bypass)
                nc.vector.tensor_reduce(cnt_cols[:, i: i + 1], msk_scr[:],
                                        axis=AX.X, op=ALU.add)

        def img_reduce(cols_ap, width, tag):
            """[128, width] -> [4, width] per-image sum via PE indicator matmul."""
            pr = psum_sm.tile([P, 8], f32, name="sm", tag="sm")
            nc.tensor.matmul(pr[:B_LOC, 0:width], c_ind4[:], cols_ap,
                             start=True, stop=True)
            out = tiny.tile([B_LOC, width], f32, name=tag, tag=tag)
            nc.vector.tensor_copy(out[:], pr[:B_LOC, 0:width])
            return out

        # initial bracket counts
        tlh = bcast(lohi[:], 6, "tlh")
        cnt6 = persist.tile([P, 8], f32, name="cnt6", tag="cnt6")
        for i in range(3):
            nc.vector.tensor_scalar(msk_scr[:], e_sb[i][:], tlh[:, i: i + 1], None,
                                    ALU.is_ge, ALU.bypass)
            nc.vector.tensor_reduce(cnt6[:, i: i + 1], msk_scr[:],
                                    axis=AX.X, op=ALU.add)
            nc.vector.tensor_scalar(msk_scr[:], e_sb[i][:], tlh[:, 3 + i: 4 + i], None,
                                    ALU.is_ge, ALU.bypass)
            nc.vector.tensor_reduce(cnt6[:, 3 + i: 4 + i], msk_scr[:],
                                    axis=AX.X, op=ALU.add)
        clh = img_reduce(cnt6[:, 0:6], 6, "clh")

        lo = tiny.tile([B_LOC, 3], f32, name="lo", tag="lo")
        nc.vector.tensor_copy(lo[:], lohi[:, 0:3])
        hi = tiny.tile([B_LOC, 3], f32, name="hi", tag="hi")
        nc.vector.tensor_copy(hi[:], lohi[:, 3:6])
        clo = tiny.tile([B_LOC, 3], f32, name="clo", tag="clo")
        nc.vector.tensor_copy(clo[:], clh[:, 0:3])
        chi = tiny.tile([B_LOC, 3], f32, name="chi", tag="chi")
        nc.vector.tensor_copy(chi[:], clh[:, 3:6])

        # ---- regula-falsi iterations ----
        for r in range(R_ITERS):
            nm = tiny.tile([B_LOC, 3], f32, name="nm", tag="nm")
            nc.vector.tensor_scalar(nm[:], clo[:], k, None, ALU.subtract, ALU.bypass)
            dn = tiny.tile([B_LOC, 3], f32, name="dn", tag="dn")
            nc.vector.tensor_sub(dn[:], clo[:], chi[:])
            dnc = tiny.tile([B_LOC, 3], f32, name="dnc", tag="dnc")
            nc.vector.tensor_scalar_max(dnc[:], dn[:], 0.75)
            rdn = tiny.tile([B_LOC, 3], f32, name="rdn", tag="rdn")
            nc.vector.reciprocal(rdn[:], dnc[:])
            rat = tiny.tile([B_LOC, 3], f32, name="rat", tag="rat")
            nc.vector.tensor_mul(rat[:], nm[:], rdn[:])
            df = tiny.tile([B_LOC, 3], f32, name="df", tag="df")
            nc.vector.tensor_sub(df[:], hi[:], lo[:])
            stp = tiny.tile([B_LOC, 3], f32, name="stp", tag="stp")
            nc.vector.tensor_mul(stp[:], df[:], rat[:])
            t_r = tiny.tile([B_LOC, 3], f32, name="t_r", tag="t_r")
            nc.vector.tensor_add(t_r[:], lo[:], stp[:])
            t_c1 = tiny.tile([B_LOC, 3], f32, name="t_c1", tag="t_c1")
            nc.vector.tensor_max(t_c1[:], t_r[:], lo[:])
            t_c = tiny.tile([B_LOC, 3], f32, name="t_c", tag="t_c")
            nc.vector.tensor_tensor(t_c[:], t_c1[:], hi[:], ALU.min)

            tcol = bcast(t_c[:], 3, "tcol")
            cntc = persist.tile([P, 8], f32, name="cntc", tag="cntc")
            count3(tcol, cntc)
            c_r = img_reduce(cntc[:, 0:3], 3, "c_r")

            ge = tiny.tile([B_LOC, 3], mybir.dt.uint8, name="ge", tag="ge")
            nc.vector.tensor_scalar(ge[:], c_r[:], k, None, ALU.is_ge, ALU.bypass)
            lo2 = tiny.tile([B_LOC, 3], f32, name="lo", tag="lo")
            nc.vector.select(lo2[:], ge[:], t_c[:], lo[:])
            clo2 = tiny.tile([B_LOC, 3], f32, name="clo", tag="clo")
            nc.vector.select(clo2[:], ge[:], c_r[:], clo[:])
            hi2 = tiny.tile([B_LOC, 3], f32, name="hi", tag="hi")
            nc.vector.select(hi2[:], ge[:], hi[:], t_c[:])
            chi2 = tiny.tile([B_LOC, 3], f32, name="chi", tag="chi")
            nc.vector.select(chi2[:], ge[:], chi[:], c_r[:])
            lo, clo, hi, chi = lo2, clo2, hi2, chi2

        # ---- final threshold: lo if (clo-k) <= (k-chi) else hi ----
        ssum = tiny.tile([B_LOC, 3], f32, name="ssum", tag="ssum")
        nc.vector.tensor_add(ssum[:], clo[:], chi[:])
        sgt = tiny.tile([B_LOC, 3], mybir.dt.uint8, name="sgt", tag="sgt")
        nc.vector.tensor_scalar(sgt[:], ssum[:], 2.0 * k, None, ALU.is_gt, ALU.bypass)
        tfin = tiny.tile([B_LOC, 3], f32, name="tfin", tag="tfin")
        nc.vector.select(tfin[:], sgt[:], hi[:], lo[:])
        tfc = bcast(tfin[:], 3, "tfc")

        # final achieved counts (debug)
        cntf = persist.tile([P, 8], f32, name="cntf", tag="cntf")
        count3(tfc, cntf)
        c_f = img_reduce(cntf[:, 0:3], 3, "c_f")

        # ---- final loss pass ----
        m0 = scratch.tile([P, FREE], f32, name="m0", tag="m0")
        nc.vector.tensor_scalar(m0[:], e_sb[0][:], tfc[:, 0:1], None,
                                ALU.is_ge, ALU.bypass)
        m1 = scratch.tile([P, FREE], f32, name="m1", tag="m1")
        nc.vector.tensor_scalar(m1[:], e_sb[1][:], tfc[:, 1:2], None,
                                ALU.is_ge, ALU.bypass)
        a1 = scratch.tile([P, FREE], f32, name="a1", tag="a1")
        nc.scalar.mul(a1[:], d_sb[1][:], w)
        a2 = scratch.tile([P, FREE], f32, name="a2", tag="a2")
        nc.scalar.mul(a2[:], d_sb[2][:], w)

        lcol = persist.tile([P, 4], f32, name="lcol", tag="lcol")
        # col0 = 2*S2_0 + S2_1
        nc.vector.scalar_tensor_tensor(lcol[:, 0:1], stats[:, 3:4], 2.0,
                                       stats[:, 4:5], ALU.mult, ALU.add)
        pairs = [(0, m0, a1), (0, m0, a2), (1, m1, a2)]
        for q, (i, mk, aj) in enumerate(pairs):
            bq = scratch.tile([P, FREE], f32, name="bq", tag="bq")
            nc.vector.scalar_tensor_tensor(bq[:], d_sb[i][:], -2.0, aj[:],
                                           ALU.mult, ALU.add)
            mb = scratch.tile([P, FREE], f32, name="mb", tag="mb")
            nc.vector.tensor_mul(mb[:], mk[:], bq[:])
            nc.vector.tensor_mul(sq_scr[:], mb[:], aj[:])
            nc.vector.tensor_reduce(lcol[:, 1 + q: 2 + q], sq_scr[:],
                                    axis=AX.X, op=ALU.add)

        ltot = persist.tile([P, 1], f32, name="ltot", tag="ltot")
        nc.vector.tensor_reduce(ltot[:], lcol[:], axis=AX.X, op=ALU.add)
        ps_fin = psum_sm.tile([P, 8], f32, name="sm", tag="sm")
        nc.tensor.matmul(ps_fin[:1, 0:1], c_ones[:], ltot[:], start=True, stop=True)
        lsb = tiny.tile([1, 1], f32, name="lsb", tag="lsb")
        nc.vector.tensor_copy(lsb[:], ps_fin[:1, 0:1])
        nc.sync.dma_start(loss_d[:], lsb[:])

        # ---- debug block [4, 24] ----
        dbg = tiny.tile([B_LOC, 24], f32, name="dbg", tag="dbg")
        for cix, src in enumerate([mu, sig, clo, chi, tfin, c_f, lo, hi]):
            nc.vector.tensor_copy(dbg[:, 3 * cix: 3 * (cix + 1)], src[:])
        nc.sync.dma_start(dbg_d[:], dbg[:])

    nc.compile()
    return nc


def _get_nc(num, weight):
    key = (num, round(float(weight), 9), POOL_PREC, R_ITERS)
    if key not in _CACHE:
        _CACHE[key] = _build(num, weight)
    return _CACHE[key]


def _pool_numpy(gt):
    g = gt.reshape(-1, C, H, SIZE, W, SIZE).sum(axis=(3, 5), dtype=np.float64)
    return g.reshape(g.shape[0], -1).astype(np.float32)


def _kernel_numpy_no_topk(out0, out1, out2, gt_density):
    outs = [o.reshape(B, -1).astype(np.float32) for o in (out0, out1, out2)]
    dmap = _pool_numpy(gt_density.reshape(B, GH, GW))
    loss = np.float64(0.0)
    for o in outs:
        loss += np.sum((o.astype(np.float64) - dmap.astype(np.float64)) ** 2)
    return np.float32(loss)


def make_in_maps(out0, out1, out2, gt_density):
    """Shard FULL inputs into per-core input maps."""
    ind4, bcast4, ones1, ind96 = _host_consts()
    o = [np.ascontiguousarray(np.asarray(x, np.float32).reshape(B, HW))
         for x in (out0, out1, out2)]
    g = np.ascontiguousarray(np.asarray(gt_density, np.float32).reshape(B * GH, GW))
    in_maps = []
    for cid in range(N_CORES):
        sl = slice(cid * B_LOC, (cid + 1) * B_LOC)
        m = {
            "gt": g[cid * B_LOC * GH: (cid + 1) * B_LOC * GH],
            "ind4": ind4, "bcast4": bcast4, "ones1": ones1, "ind96": ind96,
        }
        for i in range(3):
            m[f"out{i}"] = np.ascontiguousarray(o[i][sl].reshape(P, FREE))
        in_maps.append(m)
    return in_maps


def kernel(out0, out1, out2, gt_density, process):
    process = float(np.asarray(process))
    num = int(H * W * MAX_NOISY_RATIO * process)
    weight = MAX_WEIGHT_RATIO * process
    if num < 1:
        return _kernel_numpy_no_topk(out0, out1, out2, gt_density)

    from concourse.bass_utils import run_bass_kernel_spmd

    nc = _get_nc(num, weight)
    in_maps = make_in_maps(out0, out1, out2, gt_density)
    res = run_bass_kernel_spmd(nc, in_maps, list(range(N_CORES)))
    total = np.float64(0.0)
    for r in res.results:
        total += np.float64(r["loss"][0, 0])
    return np.float32(total)
